# revision 16
# baseline (speedup 1.0000x reference)
"""Trainium2 Bass kernel for nn_AtenMmQuint8: quint8 dense matmul.

    out = ((x - 65) * 0.199) @ ((y - 160) * 0.0215)
    x: [2048, 4096] int32 (quint8 values 0..255)
    y: [4096, 2048] int32 (quint8 values 0..255)
    out: [2048, 2048] fp32

Sharding: 4x2 tensor-parallel grid over the 8 NeuronCores (4 M-blocks x
2 N-blocks). This halves per-core HBM traffic vs. the 1x8 column-only
split (x-slice + y-slice = 24 MiB int32 -> 6 MiB as quint8 bytes).

Host staging: the inputs are quint8 tensors boxed in int32; we stage them
to the device in their natural 1-byte storage, and stage x K-major
(transposed) so the PE's stationary operand needs no on-chip transpose
(DMA transpose only supports 2-byte dtypes).

Device kernel (identical SPMD program on all 8 cores):
  - DMA k-chunks of xT (u8 [K,512]) and y (u8 [K,1024]) into SBUF.
  - Dequant bias: bf16 <- u8 + (-zero_point) on ScalarE (x) / VectorE (y).
    (q - zp) is an integer in [-160, 190] -> exactly representable in bf16.
  - PE matmul bf16 x bf16 -> fp32, accumulating the full 512x1024 output
    block across all 8 PSUM banks with a k-outer loop so the PE never
    waits on a full K pass.
  - Final copy PSUM -> SBUF fused with the combined scale (0.199*0.0215)
    on ScalarE, then DMA the fp32 block out.
"""

import numpy as np

import concourse.bass as bass
import concourse.mybir as mybir
import concourse.tile as tile
from concourse import bacc
from concourse.bass_utils import run_bass_kernel_spmd

X_ZP, Y_ZP = 65.0, 160.0
SCALE = 0.199 * 0.0215

M, K, N = 2048, 4096, 2048
GM, GN = 4, 2  # core grid: 4 M-blocks x 2 N-blocks
MC, NC = M // GM, N // GN  # 512 x 1024 per-core output block
P = 128  # partitions / k-tile size
NB = 512  # psum bank free size (one fp32 bank)
# k-tiles loaded per DMA chunk; small leading chunks get the PE started
# early, big trailing chunks amortize DMA overhead. Sums to K // P = 32.
CHUNKS = [1, 1, 2, 4, 8, 8, 8]


def _emit(tc, xT, ys, out, chunks, n_warm=24):
    """Emit the per-core device program.

    xT: [k, mc] u8 DRAM (x slice, K-major), ys: [k, nnc] u8 DRAM,
    out: [mc, nnc] fp32 DRAM.
    """
    nc = tc.nc
    k, mc = xT.shape
    nnc = ys.shape[1]
    kt = k // P
    mt = mc // P
    nt = nnc // NB
    assert sum(chunks) == kt
    cmax = max(chunks)

    fp32 = mybir.dt.float32
    bf16 = mybir.dt.bfloat16
    u8 = mybir.dt.uint8

    if True:
        with (
            tc.tile_pool(name="sb", bufs=1) as sbp,
            tc.tile_pool(name="osb", bufs=mt * nt, space="SBUF") as osbp,
            tc.tile_pool(name="ps", bufs=mt * nt, space="PSUM") as psp,
        ):
            # Everything is persistent (fits in SBUF at this problem size):
            # each chunk DMA / cast writes a disjoint slice, so no
            # instruction ever needs more than one sync wait (the ISA
            # allows only one per instruction).
            xu = sbp.tile([P, kt, mc], u8, name="xu")
            yu = sbp.tile([P, kt, nnc], u8, name="yu")
            xba = sbp.tile([P, kt, mc], bf16, name="xba")
            yba = sbp.tile([P, kt, nnc], bf16, name="yba")
            wt = sbp.tile([P, 256], bf16, name="wt")
            psum = [
                [
                    psp.tile([P, NB], fp32, tag="ps", name=f"ps_{m}_{n}")
                    for n in range(nt)
                ]
                for m in range(mt)
            ]
            # HAM prewarm: the PE sits idle for ~5 us while the first
            # chunks load+cast; run throwaway matmuls so the clock gate
            # reaches 8/8 before the real stream starts (saves the ~2 us
            # cold-rate penalty on the first ~3.4 us of real matmuls).
            nc.gpsimd.memset(wt[:], 0.0)
            for _ in range(n_warm):
                nc.tensor.matmul(
                    psum[0][0][:, :256], wt[:, :128], wt[:], start=True, stop=True
                )
            k0 = 0
            for ci, nk in enumerate(chunks):
                nc.sync.dma_start(
                    xu[:, k0 : k0 + nk, :],
                    xT[k0 * P : (k0 + nk) * P, :].rearrange("(j p) m -> p j m", p=P),
                )
                # y-loads issue from the ACT HWDGE ring so they don't
                # serialize behind the x-load issues on the SP ring
                # (each HWDGE dma_start occupies its sequencer ~0.65 us).
                nc.scalar.dma_start(
                    yu[:, k0 : k0 + nk, :],
                    ys[k0 * P : (k0 + nk) * P, :].rearrange("(j p) n -> p j n", p=P),
                )
                xb = xba[:, k0 : k0 + nk, :]
                yb = yba[:, k0 : k0 + nk, :]
                nc.vector.tensor_scalar_add(xb, xu[:, k0 : k0 + nk, :], -X_ZP)
                nc.vector.tensor_scalar_add(yb, yu[:, k0 : k0 + nk, :], -Y_ZP)

                last = ci == len(chunks) - 1
                if not last:
                    # k-outer: touch every psum bank each k-tile so the PE
                    # stream stays dense while chunks arrive.
                    for j in range(nk):
                        for m in range(mt):
                            for n in range(nt):
                                nc.tensor.matmul(
                                    psum[m][n][:],
                                    xb[:, j, m * P : (m + 1) * P],
                                    yb[:, j, n * NB : (n + 1) * NB],
                                    start=(k0 + j == 0),
                                    stop=False,
                                )
                else:
                    # m-outer in the final chunk: bank group m finishes its
                    # K accumulation early so its copy+store overlaps the
                    # remaining matmuls instead of serializing at the end.
                    for m in range(mt):
                        for j in range(nk):
                            for n in range(nt):
                                nc.tensor.matmul(
                                    psum[m][n][:],
                                    xb[:, j, m * P : (m + 1) * P],
                                    yb[:, j, n * NB : (n + 1) * NB],
                                    start=(k0 + j == 0),
                                    stop=(j == nk - 1),
                                )
                k0 += nk

            # Scale+copy PSUM->SBUF on DVE (keeps ACT entirely DMA-issue,
            # avoiding its activation-table load), one store DMA per
            # 128-row group (contiguous in `out`, halves store-issue cost).
            for m in range(mt):
                osb = osbp.tile([P, nnc], fp32, tag="osb", name=f"osb_{m}")
                for n in range(nt):
                    nc.vector.tensor_scalar_mul(
                        osb[:, n * NB : (n + 1) * NB], psum[m][n][:], SCALE
                    )
                nc.sync.dma_start(out[m * P : (m + 1) * P, :], osb[:])


def _build_nc(k=K, mc=MC, nnc=NC, chunks=CHUNKS):
    nc = bacc.Bacc("TRN2", target_bir_lowering=False, debug=False)
    xT = nc.declare_dram_parameter("xT", [k, mc], mybir.dt.uint8, isOutput=False)
    ys = nc.declare_dram_parameter("ys", [k, nnc], mybir.dt.uint8, isOutput=False)
    out = nc.declare_dram_parameter("out", [mc, nnc], mybir.dt.float32, isOutput=True)
    with tile.TileContext(nc) as tc:
        _emit(tc, xT[:], ys[:], out[:], chunks)
    nc.compile()
    return nc


_CACHE = {}


def _get_nc():
    if "nc" not in _CACHE:
        _CACHE["nc"] = _build_nc()
    return _CACHE["nc"]


def kernel(x, y):
    x = np.asarray(x)
    y = np.asarray(y)
    assert x.shape == (M, K) and y.shape == (K, N)
    # quint8 payload boxed in int32 (guaranteed 0..255 by the problem spec);
    # stage in natural 1-byte storage, x in K-major layout.
    xT_u8 = x.T.astype(np.uint8)
    y_u8 = y.astype(np.uint8)

    in_maps = []
    for i in range(GM * GN):
        mi, ni = divmod(i, GN)
        in_maps.append(
            {
                "xT": np.ascontiguousarray(xT_u8[:, mi * MC : (mi + 1) * MC]),
                "ys": np.ascontiguousarray(y_u8[:, ni * NC : (ni + 1) * NC]),
            }
        )

    res = run_bass_kernel_spmd(_get_nc(), in_maps, list(range(GM * GN)))
    _CACHE["last_results"] = res

    out = np.empty((M, N), np.float32)
    for i in range(GM * GN):
        mi, ni = divmod(i, GN)
        out[mi * MC : (mi + 1) * MC, ni * NC : (ni + 1) * NC] = res.results[i]["out"]
    return out
